# revision 26
# baseline (speedup 1.0000x reference)
"""Trainium2 Bass kernel for nn_AppPreUserPGtrDocAttn (sparse_attention).

Strategy: pure data-parallel over the window dim N across 8 NeuronCores.
Each core computes 512 output windows (last core: 509 real + 3 discarded).
All weights are replicated; inputs are sharded/padded/transposed/swizzled
on host so every DMA moves long contiguous per-partition runs.

Per-core pipeline (feature-major / transposed activations):
  A: xT[0:256, :]  = emb_app_w.T @ app_shard.T, K=10112 streamed fp8.
     The first 6 k-tiles are plain matmuls (their ~94% PE-activity
     stream releases the HAM clock gate from its cold 1.2GHz state);
     the rest use fp8 DoubleRow pairs (~1.6x measured on this part).
     xT[256:320,:] = emb_tim_w.T @ onehot(tim) one-hot gather matmul.
  B: s = attn_W.T @ xT in one 2-bank psum tile; H[f] = tanh(s[f:f+512])
     via 4 shifted ACT reads. |H| via ACT Abs.
  C/D: DVE/Pool compute only the 8 H[f]*y[n+f] products (y read straight
     from 2-bank psum / an SBUF copy); all reductions (l1 = sum|H|,
     pooled = sum_f prod) run on the tensor engine as identity-matmul
     PSUM accumulation. o2a = pooled * uid_emb * 64 -> fp8.
  E: the 1/L1 attention normalization is folded into stage F's sigmoid
     scale (per-psum-partition = per-window), obtained by PE-transposing
     L1; the ptim one-hot/bias plane is pre-multiplied by L1 so that
     rec*L1 = 1 leaves ptim and bias contributions unscaled.
  F: logits = o2a.T @ decw + onehot128(ptim,bias) @ t2b, where t2b is a
     host-folded [128,10000] table (emb_tim_w @ dec_w[:,256:].T * 1024
     plus all bias terms). Three uniform 128-row fp8 matmuls per psum
     bank (uniform tile size keeps the PE pipelined at ~216ns/matmul;
     DoubleRow does not double-pump on this part and mixing tile sizes
     flushes the pipeline). score = sigmoid(logits * rec/1024) via one
     4-bank-wide ACT per 2048 cols, written bf16 (10000 real cols).

DMA: appT/wapp/decw/t2b host-swizzled to [128, *] row-major so every
transfer is 2-4KB contiguous per partition; decw/t2b deferred behind the
stage-A stream; first app batches issued before const loads.
"""

import numpy as np

try:
    import concourse.bass as bass
except ImportError:  # pragma: no cover
    import sys

    sys.path.insert(0, "/opt/trn_rl_repo")
    import concourse.bass as bass

import ml_dtypes

import concourse.mybir as mybir
from concourse import bacc, bass_utils
from concourse import tile
from concourse.tile import TileContext

BF = ml_dtypes.bfloat16
F32 = mybir.dt.float32
BF16 = mybir.dt.bfloat16
FP8 = mybir.dt.float8e4
F8 = ml_dtypes.float8_e4m3
AF = mybir.ActivationFunctionType
ALU = mybir.AluOpType
DR = mybir.MatmulPerfMode.DoubleRow

S = 4096            # sequence length
NWIN = S - 3        # 4093 windows
NCORES = 8
R = 512             # windows per core (last core: 509 real)
RH = R + 3          # x rows needed per core (halo)
RP = 520            # padded col count for xT/appT (512 + 8)
KAPP = 10000        # app vocab / contraction dim
KAPPP = 10112       # padded to 79 k-tiles of 128
NKT = KAPPP // 128  # 79
KB = 8              # k-tiles per DMA batch
E = 256             # app emb dim
TE = 64             # tim emb dim
D = 320             # INPUT_SIZE
NOUT = 10000        # decoder outputs (unpadded in free dim)
# stage-F column groups: 4x2048 + 1024 + 784 = 10000 (small tail
# group shortens the exposed final ACT+DMA)
FGROUPS = [(0, 2048), (2048, 2048), (4096, 2048), (6144, 2048),
           (8192, 1024), (9216, 784)]

_CACHE: dict = {}


def _build():
    nc = bacc.Bacc()

    appT_d = nc.declare_dram_parameter("appT", [128, NKT * RP], FP8, isOutput=False)
    wapp_d = nc.declare_dram_parameter("wapp", [128, NKT * E], FP8, isOutput=False)
    decw_d = nc.declare_dram_parameter("decw", [128, 2 * NOUT], FP8, isOutput=False)
    t2b_d = nc.declare_dram_parameter("t2b", [128, NOUT], FP8, isOutput=False)
    fcw_d = nc.declare_dram_parameter("fcw", [128, 3 * E], BF16, isOutput=False)
    attnwr_d = nc.declare_dram_parameter("attnwr", [128, 3 * 128], BF16, isOutput=False)
    embt_d = nc.declare_dram_parameter("embt", [48, TE], BF16, isOutput=False)
    timv_d = nc.declare_dram_parameter("timv", [1, RP], BF16, isOutput=False)
    ptimv_d = nc.declare_dram_parameter("ptimv", [1, R], BF16, isOutput=False)
    # constf cols: 0 iota, 1:3 uid_emb*64, 3:5 fc_b, 5:9 attn_b
    constf_d = nc.declare_dram_parameter("constf", [128, 10], F32, isOutput=False)
    ident_d = nc.declare_dram_parameter("ident", [128, 128], F32, isOutput=False)
    out_d = nc.declare_dram_parameter("out", [R, NOUT], BF16, isOutput=True)

    with TileContext(nc) as tc:
        with (
            tc.tile_pool(name="const", bufs=1) as const,
            tc.tile_pool(name="sb", bufs=1) as sb,
            tc.tile_pool(name="apool", bufs=4) as apool,
            tc.tile_pool(name="wpool", bufs=4) as wpool,
            tc.tile_pool(name="dpool", bufs=3) as dpool,
            tc.tile_pool(name="opool", bufs=3) as opool,
            tc.tile_pool(name="tmp", bufs=2) as tmp,
        ):
            # ---- first app/wapp batches: issued before everything else so
            #      the stage-A stream starts with no dead time ----
            NWARM = 4
            BATCHES = [2, 2, 4, 8, 16, 16, 16, 15]
            MAXB = max(BATCHES)
            assert sum(BATCHES) == NKT
            appT_r = appT_d.rearrange("p (kt c) -> p kt c", c=RP)
            wapp_r = wapp_d.rearrange("p (kt e) -> p kt e", e=E)
            pre = []
            kt0 = 0
            for nb in BATCHES[:2]:
                at = apool.tile([128, MAXB, RP], FP8, name="at")
                wt = wpool.tile([128, MAXB, E], FP8, name="wt")
                at_dma = nc.sync.dma_start(at[:, 0:nb, :],
                                           appT_r[:, kt0:kt0 + nb, :])
                nc.sync.dma_start(wt[:, 0:nb, :], wapp_r[:, kt0:kt0 + nb, :])
                pre.append((at, wt, at_dma))
                kt0 += nb

            # ---- constants / small inputs ----
            # PE clock-gate pre-warm operands first on the vector queue:
            # no DMA dependency, so the warm-up matmuls start right after
            # engine init and release HAM before the first app k-tile
            wstat = const.tile([128, 128], BF16)
            nc.vector.memset(wstat[:], 0.25)
            wmov = const.tile([128, 512], BF16)
            nc.vector.memset(wmov[:], 0.25)
            ones_sb = const.tile([1, 128], BF16)
            nc.vector.memset(ones_sb[:], 1.0)
            constf_sb = const.tile([128, 10], F32)
            nc.sync.dma_start(constf_sb[:], constf_d[:, :])
            timv_sb = const.tile([1, RP], BF16)
            nc.sync.dma_start(timv_sb[:], timv_d[:, :])
            ptimv_sb = const.tile([1, R], BF16)
            nc.sync.dma_start(ptimv_sb[:], ptimv_d[:, :])
            embt_sb = const.tile([48, TE], BF16)
            nc.sync.dma_start(embt_sb[:], embt_d[:, :])
            attnwr_sb = const.tile([128, 3, 128], BF16)
            nc.sync.dma_start(attnwr_sb[:],
                              attnwr_d.rearrange("p (t m) -> p t m", t=3))
            fcw_sb = const.tile([128, 3, E], BF16)
            nc.sync.dma_start(fcw_sb[:], fcw_d.rearrange("p (t e) -> p t e", t=3))
            ident_sb = const.tile([128, 128], F32)
            nc.sync.dma_start(ident_sb[:], ident_d[:, :])
            identb_sb = const.tile([128, 128], BF16)
            nc.vector.tensor_copy(identb_sb[:], ident_sb[:])
            iota_sb = constf_sb[:, 0:1]

            # pre-warm ACT tables off the critical path
            warm = const.tile([1, 1], F32)
            nc.vector.memset(warm[:], 0.5)
            nc.scalar.activation(warm[:], warm[:], AF.Tanh)
            nc.scalar.activation(warm[:], warm[:], AF.Sigmoid)
            nc.scalar.activation(warm[:], warm[:], AF.Abs)

            # persistent activations
            xTa = sb.tile([128, 2, RP], BF16)      # x.T features 0:256
            xTt = sb.tile([TE, RP], BF16)          # x.T features 256:320
            H_b = sb.tile([128, 4, R], BF16)       # tanh windows, bcast over P
            Habs = sb.tile([128, 4, R], BF16)      # |H|
            l1sb = sb.tile([128, R], F32)          # sum_f |H|, bcast over P
            recT4 = sb.tile([128, 4], F32)         # per-window 1/(1024*L1)
            yT1 = sb.tile([128, RP], F32)          # mt1 fc product (for Pool)
            prods = sb.tile([128, 8, R], BF16)     # H[f]*y products, (mt,f)
            o2a = sb.tile([128, 2, R], FP8)        # out2.T rows 0:256, x64
            ohp_raw = sb.tile([TE, R], F32)        # onehot(ptim)+bias rows
            ohp64 = sb.tile([128, R], FP8)         # same, scaled by L1; rows
                                                   # 64:128 zero (uniform
                                                   # 128-row stage-F tiles)
            t2b_sb = const.tile([128, NOUT], FP8)

            # ---- preamble: tim/ptim one-hot gathers (overlap stage-A DMA) --
            with (
                tc.tile_pool(name="psT", bufs=1, space="PSUM") as psT,
                tc.tile_pool(name="psA", bufs=1, space="PSUM") as psA,
            ):
                pb = psT.tile([48, 512], F32)
                pt = psT.tile([TE, 512], F32)
                pp = psT.tile([TE, 512], F32)
                pwarm = psT.tile([128, 512], F32)
                oh = tmp.tile([48, RP], BF16, name="oh")
                for i in range(15):
                    nc.tensor.matmul(pwarm[:], wstat[:], wmov[:],
                                     start=True, stop=True,
                                     skip_group_check=True)

                nc.tensor.matmul(pb[:], ones_sb[0:1, 0:48], timv_sb[0:1, 0:512],
                                 start=True, stop=True)
                nc.vector.tensor_scalar(oh[:, 0:512], pb[:], iota_sb[0:48, :],
                                        None, op0=ALU.is_equal)
                nc.tensor.matmul(pb[0:48, 0:8], ones_sb[0:1, 0:48],
                                 timv_sb[0:1, 512:RP], start=True, stop=True)
                nc.vector.tensor_scalar(oh[:, 512:RP], pb[0:48, 0:8],
                                        iota_sb[0:48, :], None, op0=ALU.is_equal)
                nc.tensor.matmul(pt[:], embt_sb[:], oh[:, 0:512],
                                 start=True, stop=True)
                nc.vector.tensor_copy(xTt[:, 0:512], pt[:])
                nc.tensor.matmul(pt[0:TE, 0:8], embt_sb[:], oh[:, 512:RP],
                                 start=True, stop=True)
                nc.vector.tensor_copy(xTt[:, 512:RP], pt[0:TE, 0:8])

                # ptim one-hot (64 rows: 48 one-hot + always-on bias row 48;
                # rows 49:63 compare false -> 0; row 48 gets +1 via col 9)
                nc.tensor.matmul(pp[:], ones_sb[0:1, 0:TE], ptimv_sb[:],
                                 start=True, stop=True)
                nc.vector.tensor_scalar(ohp_raw[:], pp[0:TE, :],
                                        iota_sb[0:TE, :],
                                        constf_sb[0:TE, 9:10],
                                        op0=ALU.is_equal, op1=ALU.add)
                nc.vector.memset(ohp64[TE:128, :], 0.0)

                # ---- stage A: xT[0:256] = wapp.T @ appT, fp8 DoubleRow ----
                # first NWARM k-tiles run as plain fp8 matmuls: their ~94%
                # PE-activity stream releases the HAM clock gate (cold
                # DoubleRow sits below the release threshold and would stay
                # stuck at 1.2GHz); the rest run as DoubleRow pairs.
                pxa0 = psA.tile([128, 512], F32)
                pxa1 = psA.tile([128, 512], F32)
                px80 = psA.tile([128, 8], F32)
                px81 = psA.tile([128, 8], F32)
                pxa = [pxa0, pxa1]
                px8 = [px80, px81]
                kt0 = 0
                at_dma_by_batch = []
                for bi, nb in enumerate(BATCHES):
                    if bi < 2:
                        at, wt, at_dma = pre[bi]
                    else:
                        at = apool.tile([128, MAXB, RP], FP8, name="at")
                        wt = wpool.tile([128, MAXB, E], FP8, name="wt")
                        at_dma = nc.sync.dma_start(at[:, 0:nb, :],
                                                   appT_r[:, kt0:kt0 + nb, :])
                        nc.sync.dma_start(wt[:, 0:nb, :],
                                          wapp_r[:, kt0:kt0 + nb, :])
                    at_dma_by_batch.append(at_dma)
                    at_last, wt_last = at, wt
                    if kt0 < NWARM:
                        for k in range(nb):
                            kt = kt0 + k
                            start = kt == 0
                            for mt in range(2):
                                lhsT = wt[:, k, mt * 128:(mt + 1) * 128]
                                nc.tensor.matmul(pxa[mt][:], lhsT,
                                                 at[:, k, 0:512],
                                                 start=start, stop=False)
                                nc.tensor.matmul(px8[mt][:], lhsT,
                                                 at[:, k, 512:RP],
                                                 start=start, stop=False)
                    else:
                        for k in range(0, nb, 2):
                            kt = kt0 + k
                            if k == nb - 1 and kt == NKT - 1:
                                # odd tail tile: plain matmul closes the group
                                for mt in range(2):
                                    lhsT = wt[:, k, mt * 128:(mt + 1) * 128]
                                    nc.tensor.matmul(pxa[mt][:], lhsT,
                                                     at[:, k, 0:512],
                                                     start=False, stop=True)
                                    nc.tensor.matmul(px8[mt][:], lhsT,
                                                     at[:, k, 512:RP],
                                                     start=False, stop=True)
                                continue
                            for mt in range(2):
                                lhsT = wt[:, k:k + 2, mt * 128:(mt + 1) * 128]
                                nc.tensor.matmul(pxa[mt][:], lhsT,
                                                 at[:, k:k + 2, 0:512],
                                                 start=False, stop=False,
                                                 perf_mode=DR)
                                nc.tensor.matmul(px8[mt][:], lhsT,
                                                 at[:, k:k + 2, 512:RP],
                                                 start=False, stop=False,
                                                 perf_mode=DR)
                    kt0 += nb
                # deferred big loads: start them near the end of the A stream
                t2b_dma = nc.sync.dma_start(t2b_sb[:], t2b_d[:, :])
                tile.add_dep_helper(t2b_dma.ins, at_dma_by_batch[6].ins,
                                    sync=True,
                                    reason="defer t2b until A stream tail")
                nc.vector.tensor_scalar_mul(xTa[:, 0, 0:512], pxa0[:],
                                            1.0 / 16.0)
                nc.vector.tensor_scalar_mul(xTa[:, 0, 512:RP], px80[:],
                                            1.0 / 16.0)
                nc.vector.tensor_scalar_mul(xTa[:, 1, 0:512], pxa1[:],
                                            1.0 / 16.0)
                nc.vector.tensor_scalar_mul(xTa[:, 1, 512:RP], px81[:],
                                            1.0 / 16.0)

            # ---- stage B/C/D/E ----
            # Score vector s in one 2-bank psum tile; tanh ACTs read shifted
            # windows. DVE/Pool compute only the 8 H*y products; all
            # summation (l1 = sum|H| and pooled = sum_f prod_f) runs on the
            # tensor engine as identity-matmul PSUM accumulation, which also
            # keeps the HAM clock gate released through the middle. The 1/L1
            # normalization is folded into stage F's sigmoid scale; the
            # one-hot/bias plane is pre-multiplied by L1 so rec*L1 = 1 leaves
            # ptim and bias contributions unscaled.
            with tc.tile_pool(name="psB", bufs=1, space="PSUM") as psB:
                py20 = psB.tile([128, 2, 512], F32, name="py20")
                py21 = psB.tile([128, 2, 512], F32, name="py21")
                py2f0 = py20.rearrange("p b c -> p (b c)")
                py2f1 = py21.rearrange("p b c -> p (b c)")
                pw2 = psB.tile([128, 2, 512], F32, name="pw2")
                pw2f = pw2.rearrange("p b c -> p (b c)")
                po2 = psB.tile([128, 2, 512], F32, name="po2")
                xts = [xTa[:, 0, :], xTa[:, 1, :], xTt[:, :]]
                klens = [128, 128, TE]
                for kt in range(3):
                    xt, kl = xts[kt], klens[kt]
                    st, sp = kt == 0, kt == 2
                    nc.tensor.matmul(pw2[:, 0, :], attnwr_sb[0:kl, kt, :],
                                     xt[0:kl, 0:512], start=st, stop=sp)
                    nc.tensor.matmul(pw2[:, 1, 0:8], attnwr_sb[0:kl, kt, :],
                                     xt[0:kl, 512:RP], start=st, stop=sp)
                    for mt, pyt in ((0, py20), (1, py21)):
                        lhsT = fcw_sb[0:kl, kt, mt * 128:(mt + 1) * 128]
                        nc.tensor.matmul(pyt[:, 0, :], lhsT, xt[0:kl, 0:512],
                                         start=st, stop=sp)
                        nc.tensor.matmul(pyt[:, 1, 0:8], lhsT,
                                         xt[0:kl, 512:RP], start=st, stop=sp)
                # HAM bridge: PE work gated only on xTa (ready now), so the
                # PE never sees an idle window while ACT computes tanh/|H|
                for i in range(10):
                    nc.tensor.matmul(po2[:, 0, :], wt_last[:, 0:2, 0:128],
                                     at_last[:, 0:2, 0:512], start=True,
                                     stop=True, perf_mode=DR,
                                     skip_group_check=True)
                # ACT: the 4 shifted tanh windows, then stage the mt1 fc
                # product to SBUF (Pool cannot read PSUM)
                nc.scalar.activation(H_b[:, 0, :], pw2f[:, 0:R], AF.Tanh,
                                     bias=constf_sb[:, 5:6])
                nc.scalar.copy(yT1[:, 0:RP], py2f1[:, 0:RP])
                for f in range(1, 4):
                    nc.scalar.activation(H_b[:, f, :], pw2f[:, f:f + R],
                                         AF.Tanh,
                                         bias=constf_sb[:, 5 + f:6 + f])
                Hf = H_b.rearrange("p f n -> p (f n)")
                Haf = Habs.rearrange("p f n -> p (f n)")
                nc.scalar.activation(Haf[:, 0:2 * R], Hf[:, 0:2 * R], AF.Abs)
                nc.scalar.activation(Haf[:, 2 * R:4 * R], Hf[:, 2 * R:4 * R],
                                     AF.Abs)

                # products (the only elementwise heavy ops): DVE does mt0
                # from psum, Pool does mt1 from SBUF
                for f in range(4):
                    nc.vector.tensor_mul(prods[:, f, :], H_b[:, f, :],
                                         py2f0[:, f:f + R])
                    nc.gpsimd.tensor_mul(prods[:, 4 + f, :], H_b[:, f, :],
                                         yT1[:, f:f + R])

                # PE: l1 = sum_f |H_f| accumulated into pw2 bank 1 (free
                # after the tanh reads), then pooled = sum_f prod into po2
                for f in range(4):
                    nc.tensor.matmul(pw2[:, 1, :], identb_sb[:],
                                     Habs[:, f, :], start=(f == 0),
                                     stop=(f == 3))
                for mt in range(2):
                    for f in range(4):
                        nc.tensor.matmul(po2[:, mt, :], identb_sb[:],
                                         prods[:, 4 * mt + f, :],
                                         start=(f == 0), stop=(f == 3))
                nc.scalar.copy(l1sb[:], pw2[:, 1, :])
                nc.vector.tensor_mul(ohp64[0:TE, :], ohp_raw[:], l1sb[0:TE, :])
                for mt in range(2):
                    nc.vector.tensor_scalar(o2a[:, mt, :], po2[:, mt, :],
                                            constf_sb[:, 1 + mt:2 + mt], None,
                                            op0=ALU.mult)

                # per-window 1/(1024*L1) onto partitions via PE transposes
                # (pw2 bank 0 is free after the last tanh read)
                pT = pw2[:, 0, 0:128]
                for mt in range(4):
                    nc.tensor.matmul(pT, l1sb[:, mt * 128:(mt + 1) * 128],
                                     ident_sb[:], is_transpose=True,
                                     start=True, stop=True)
                    nc.vector.tensor_scalar_mul(recT4[:, mt:mt + 1],
                                                pT[:, 0:1], 1024.0)
                nc.vector.reciprocal_approx_fast(recT4[:], recT4[:])

                # HAM hold-over into stage F
                for i in range(4):
                    nc.tensor.matmul(po2[:, 0, :], wt_last[:, 0:2, 0:128],
                                     o2a[:, 0:2, 0:512], start=True,
                                     stop=True, perf_mode=DR,
                                     skip_group_check=True)

            # ---- stage F: score = sigmoid((o2a.T @ decw + ohp64.T @ t2b)/1024)
            decw_r = decw_d.rearrange("p (i v) -> p i v", i=2)
            with tc.tile_pool(name="psF", bufs=2, space="PSUM") as psF:
                for g, (c0, cw) in enumerate(FGROUPS):
                    dw = dpool.tile([128, 2, 2048], FP8, name="dw")
                    dw_dma = nc.sync.dma_start(dw[:, :, 0:cw],
                                               decw_r[:, :, c0:c0 + cw])
                    if g < 2:
                        tile.add_dep_helper(
                            dw_dma.ins, at_dma_by_batch[7].ins, sync=True,
                            reason="defer dec stream until A stream tail")
                    nsub = (cw + 511) // 512
                    for mt in range(4):
                        pf4 = psF.tile([128, 4, 512], F32, name="pf4")
                        pf_flat = pf4.rearrange("p b c -> p (b c)")
                        for sub in range(nsub):
                            ncols = min(512, cw - sub * 512)
                            cs = c0 + sub * 512
                            for kt in range(2):
                                nc.tensor.matmul(
                                    pf4[:, sub, 0:ncols],
                                    o2a[:, kt, mt * 128:(mt + 1) * 128],
                                    dw[:, kt, sub * 512:sub * 512 + ncols],
                                    start=(kt == 0), stop=False)
                            nc.tensor.matmul(
                                pf4[:, sub, 0:ncols],
                                ohp64[:, mt * 128:(mt + 1) * 128],
                                t2b_sb[:, cs:cs + ncols],
                                start=False, stop=True)
                        ob = opool.tile([128, 2048], BF16, name="ob")
                        nc.scalar.activation(ob[:, 0:cw], pf_flat[:, 0:cw],
                                             AF.Sigmoid,
                                             scale=recT4[:, mt:mt + 1])
                        nc.sync.dma_start(
                            out_d[mt * 128:(mt + 1) * 128, c0:c0 + cw],
                            ob[:, 0:cw])

    nc.finalize()
    return nc


def _host_prep(tim, app, uid, ptim, emb_tim_w, emb_uid_w, emb_app_w,
               attn_W, attn_b, attn_fc_w, attn_fc_b, dec_w, dec_b):
    """Shard + pad + transpose + swizzle + cast all inputs for 8 cores."""
    app = np.asarray(app, dtype=np.float32)
    tim = np.asarray(tim).reshape(-1)
    ptim = np.asarray(ptim).reshape(-1)
    uid = int(np.asarray(uid).reshape(-1)[0])

    app_f8 = app.astype(F8)

    wapp = np.zeros((NKT, 128, E), dtype=F8)
    wapp.reshape(-1, E)[:KAPP] = (
        np.asarray(emb_app_w, dtype=np.float32) * 16.0).astype(F8)
    wapp = np.ascontiguousarray(wapp.transpose(1, 0, 2)).reshape(128, NKT * E)

    dwT = np.ascontiguousarray(np.asarray(dec_w, dtype=np.float32).T)  # [320,10000]
    decw = (dwT[:E] * 16.0).astype(F8).reshape(2, 128, NOUT)
    decw = np.ascontiguousarray(decw.transpose(1, 0, 2)).reshape(128, 2 * NOUT)

    uide = np.asarray(emb_uid_w, dtype=np.float32)[uid]
    fcb = np.asarray(attn_fc_b, dtype=np.float32).reshape(-1)
    decb = np.asarray(dec_b, dtype=np.float32).reshape(-1)
    t2b = np.zeros((128, NOUT), dtype=np.float32)
    t2b[0:48] = np.asarray(emb_tim_w, dtype=np.float32) @ dwT[E:D]
    t2b[48] = decb + (fcb * uide) @ dwT[:E]
    t2b = np.clip(t2b * 1024.0, -448.0, 448.0).astype(F8)

    fcw = np.zeros((3, 128, E), dtype=BF)
    fcw.reshape(-1, E)[:D] = np.ascontiguousarray(
        np.asarray(attn_fc_w, dtype=np.float32).T).astype(BF)
    fcw = np.ascontiguousarray(fcw.transpose(1, 0, 2)).reshape(128, 3 * E)

    embt = np.asarray(emb_tim_w, dtype=np.float32).astype(BF)

    attnw = np.zeros((3, 128), dtype=np.float32)
    attnw.reshape(-1)[:D] = np.asarray(attn_W, dtype=np.float32).reshape(-1)
    attnwr = np.repeat(attnw.T[:, :, None], 128, axis=2).astype(BF)  # [128,3,128]
    attnwr = np.ascontiguousarray(attnwr).reshape(128, 3 * 128)

    ident = np.eye(128, dtype=np.float32)

    constf = np.zeros((128, 10), dtype=np.float32)
    constf[48, 9] = 1.0
    constf[:, 0] = np.arange(128, dtype=np.float32)
    constf[:, 1] = uide[0:128] * 64.0
    constf[:, 2] = uide[128:256] * 64.0
    constf[:, 3] = fcb[0:128]
    constf[:, 4] = fcb[128:256]
    constf[:, 5:9] = np.asarray(attn_b, dtype=np.float32).reshape(1, 4)

    in_maps = []
    for c in range(NCORES):
        r0 = c * R
        r1 = min(r0 + RH, S)
        n = r1 - r0
        appT = np.zeros((NKT, 128, RP), dtype=F8)
        appT.reshape(-1, RP)[:KAPP, :n] = app_f8[r0:r1].T
        appT = np.ascontiguousarray(appT.transpose(1, 0, 2)).reshape(128, NKT * RP)

        timv = np.zeros((1, RP), dtype=BF)
        timv[0, :n] = tim[r0:r1].astype(BF)

        ptimv = np.zeros((1, R), dtype=BF)
        np_ = min(r0 + R, NWIN) - r0
        ptimv[0, :np_] = ptim[r0:r0 + np_].astype(BF)

        in_maps.append({
            "appT": appT, "wapp": wapp, "decw": decw, "t2b": t2b,
            "fcw": fcw, "attnwr": attnwr, "embt": embt, "timv": timv,
            "ptimv": ptimv, "constf": constf, "ident": ident,
        })
    return in_maps


def kernel(tim, app, loc, uid, ptim, emb_tim_w, emb_uid_w, emb_app_w,
           attn_W, attn_b, attn_fc_w, attn_fc_b, dec_w, dec_b,
           _trace=False, _trace_kwargs=None):
    if "nc" not in _CACHE:
        _CACHE["nc"] = _build()
    nc = _CACHE["nc"]

    in_maps = _host_prep(tim, app, uid, ptim, emb_tim_w, emb_uid_w, emb_app_w,
                         attn_W, attn_b, attn_fc_w, attn_fc_b, dec_w, dec_b)

    kw = {}
    if _trace:
        kw["trace"] = True
        if _trace_kwargs:
            kw.update(_trace_kwargs)
    res = bass_utils.run_bass_kernel_spmd(nc, in_maps, core_ids=list(range(NCORES)), **kw)
    _CACHE["last_result"] = res

    outs = []
    for c in range(NCORES):
        nrows = R if c < NCORES - 1 else NWIN - (NCORES - 1) * R
        outs.append(np.asarray(res.results[c]["out"])[:nrows, :NOUT])
    return np.concatenate(outs, axis=0).astype(np.float32)
